# revision 16
# baseline (speedup 1.0000x reference)
"""Multi-head VQ codebook quantization kernel for Trainium2 (8 NeuronCores).

Problem: embeds [32768, 8, 64] f32, codebook [8, 512, 64] f32 ->
  quantized [32768, 8, 64] f32, idx [32768, 8] int32, loss scalar f32.

Strategy
--------
N-sharded data parallel: core i handles rows [i*4096, (i+1)*4096) for all 8
heads. Per core, per head, per 128-row tile:

  scores[n, k] = z[n]·c[k] - 0.5||c[k]||²  via two fp16 matmuls with exact
  fp32 PSUM accumulation (fp16 hi/lo split => fp32-class precision):
    matmul1: [z_hi; 1; 1]^T (66) @ [c_hi; b_hi; b_lo]  (b = -0.5||c||² split)
    matmul2: [z_hi; z_lo]^T (128) @ [c_lo; c_hi]
  (the dropped z_lo·c_lo term is ~2^-22 relative — below fp32 rounding)

  ACT copies PSUM->SBUF; DVE reduce_max produces the row max m; a fused DVE
  scalar_tensor_tensor computes idx = sum_k (s >= m) * iota[k] exactly.
  Per 4-head batch the indices get head offsets added (global codebook row
  ids), are converted to int16, rearranged into the wrapped-16 list layout
  via small SBUF->SBUF DMAs, and GPSIMD dma_gather fetches the codebook
  rows (the quantized output). Loss is finished on host from the returned
  row maxima: ||z - c_idx||² = ||z||² - 2 m.
"""

import numpy as np

import concourse.bacc as bacc
import concourse.mybir as mybir
from concourse.tile import TileContext
from concourse.bass_utils import run_bass_kernel_spmd

N, H, D, K = 32768, 8, 64, 512
NCORES = 8
NS = N // NCORES            # rows per core (4096)
TILES_PER_HEAD = NS // 128  # 32
TILES = H * TILES_PER_HEAD  # 256
BETA = 0.25
HB = 4                      # heads per gather batch
BATCH_POS = HB * NS         # positions per batch (16384)

_NC_CACHE = {}


def _build_bass(psum_bufs=6, s_bufs=4, loop_reps=None):
    nc = bacc.Bacc()

    za = nc.dram_tensor("za", [128, H * NS], mybir.dt.float16, kind="ExternalInput")
    zb = nc.dram_tensor("zb", [66, H * NS], mybir.dt.float16, kind="ExternalInput")
    rhs1 = nc.dram_tensor("rhs1", [H, 66, K], mybir.dt.float16, kind="ExternalInput")
    rhs2 = nc.dram_tensor("rhs2", [H, 128, K], mybir.dt.float16, kind="ExternalInput")
    iota = nc.dram_tensor("iota", [128, K], mybir.dt.float32, kind="ExternalInput")
    goff = nc.dram_tensor("goff", [128, TILES], mybir.dt.float32,
                          kind="ExternalInput")
    cbf = nc.dram_tensor("cbf", [H * K, D], mybir.dt.float32, kind="ExternalInput")

    zq_out = nc.dram_tensor("zq_out", [128, TILES * D], mybir.dt.float32,
                            kind="ExternalOutput")
    idx_out = nc.dram_tensor("idx_out", [128, TILES], mybir.dt.int16,
                             kind="ExternalOutput")
    m_out = nc.dram_tensor("m_out", [128, TILES], mybir.dt.float32,
                           kind="ExternalOutput")

    with TileContext(nc) as tc:
        with (
            tc.tile_pool(name="const", bufs=1) as constp,
            tc.tile_pool(name="acc", bufs=1) as accp,
            tc.tile_pool(name="zin", bufs=2) as zinp,
            tc.tile_pool(name="rhs", bufs=2) as rhsp,
            tc.tile_pool(name="sco", bufs=s_bufs) as scop,
            tc.tile_pool(name="scr", bufs=2) as scrp,
            tc.tile_pool(name="zq", bufs=2) as zqp,
            tc.tile_pool(name="wrp", bufs=2) as wrpp,
            tc.tile_pool(name="ps", bufs=psum_bufs, space="PSUM") as psp,
        ):
            iota_sb = constp.tile([128, K], mybir.dt.float32)
            nc.sync.dma_start(iota_sb[:, :], iota[:, :])
            goff_sb = constp.tile([128, TILES], mybir.dt.float32)
            nc.sync.dma_start(goff_sb[:, :], goff[:, :])

            mv_sb = accp.tile([128, TILES], mybir.dt.float32)
            idxf_sb = accp.tile([128, TILES], mybir.dt.float32)

            # pre-touch consts on DVE so DMA waits don't land on STT ops
            pre = accp.tile([128, 1], mybir.dt.float32)
            nc.vector.tensor_copy(pre[:, :], iota_sb[:, 0:1])
            nc.vector.tensor_copy(pre[:, :], goff_sb[:, 0:1])

            import contextlib
            loop_cm = (tc.For_i(0, loop_reps, 1) if loop_reps
                       else contextlib.nullcontext())
            with loop_cm:
                _emit_body(nc, tc, locals())
            nc.sync.dma_start(m_out[:, :], mv_sb[:, :])
    nc.finalize()
    return nc


def _emit_body(nc, tc, env):
    (za, zb, rhs1, rhs2, cbf, zq_out, idx_out,
     iota_sb, goff_sb, mv_sb, idxf_sb) = (
        env["za"], env["zb"], env["rhs1"], env["rhs2"], env["cbf"],
        env["zq_out"], env["idx_out"],
        env["iota_sb"], env["goff_sb"], env["mv_sb"], env["idxf_sb"])
    rhsp, zinp, scop, scrp, zqp, wrpp, psp = (
        env["rhsp"], env["zinp"], env["scop"], env["scrp"], env["zqp"],
        env["wrpp"], env["psp"])
    if True:
        if True:
            for h in range(H):
                rhs1_sb = rhsp.tile([66, K], mybir.dt.float16, tag="rhs1")
                rhs2_sb = rhsp.tile([128, K], mybir.dt.float16, tag="rhs2")
                nc.sync.dma_start(rhs1_sb[:, :], rhs1[h, :, :])
                nc.sync.dma_start(rhs2_sb[:, :], rhs2[h, :, :])

                col0 = h * NS
                za_sb = zinp.tile([128, NS], mybir.dt.float16, tag="za")
                zb_sb = zinp.tile([66, NS], mybir.dt.float16, tag="zb")
                nc.sync.dma_start(za_sb[:, :], za[:, col0:col0 + NS])
                nc.sync.dma_start(zb_sb[:, :], zb[:, col0:col0 + NS])

                for j in range(TILES_PER_HEAD):
                    t = h * TILES_PER_HEAD + j
                    lo = j * 128

                    ps = psp.tile([128, K], mybir.dt.float32, tag="ps")
                    nc.tensor.matmul(ps[:, :], zb_sb[:, lo:lo + 128],
                                     rhs1_sb[:, :], start=True, stop=False)
                    nc.tensor.matmul(ps[:, :], za_sb[:, lo:lo + 128],
                                     rhs2_sb[:, :], start=False, stop=True)

                    s_sb = scop.tile([128, K], mybir.dt.float32, tag="s")
                    nc.scalar.copy(s_sb[:, :], ps[:, :])

                    nc.vector.reduce_max(mv_sb[:, t:t + 1], s_sb[:, :],
                                         axis=mybir.AxisListType.X)

                    scratch = scrp.tile([128, K], mybir.dt.float32, tag="scr")
                    nc.vector.scalar_tensor_tensor(
                        out=scratch[:, :], in0=s_sb[:, :],
                        scalar=mv_sb[:, t:t + 1], in1=iota_sb[:, :],
                        op0=mybir.AluOpType.is_ge, op1=mybir.AluOpType.mult,
                        accum_out=idxf_sb[:, t:t + 1],
                    )

                # ---- gather batch epilogue after heads 3 and 7 ----
                if h % HB != HB - 1:
                    continue
                b = h // HB
                tb = b * HB * TILES_PER_HEAD          # first tile col of batch
                bw = HB * TILES_PER_HEAD              # tile cols per batch (128)

                # local idx -> global codebook row id, clamped, int16
                nc.vector.tensor_tensor(
                    out=idxf_sb[:, tb:tb + bw], in0=idxf_sb[:, tb:tb + bw],
                    in1=goff_sb[:, tb:tb + bw], op=mybir.AluOpType.add)
                nc.vector.tensor_scalar_min(
                    idxf_sb[:, tb:tb + bw], idxf_sb[:, tb:tb + bw],
                    float(H * K - 1))
                idx16 = wrpp.tile([128, bw], mybir.dt.int16, tag="idx16")
                nc.vector.tensor_copy(idx16[:, :], idxf_sb[:, tb:tb + bw])
                nc.sync.dma_start(idx_out[:, tb:tb + bw], idx16[:, :])

                # wrapped-16 list: wrap[pl, ph*bw + t] = idx16[ph*16+pl, t]
                wrap_b = wrpp.tile([128, BATCH_POS // 16], mybir.dt.int16,
                                   tag="wrap")
                for ph in range(8):
                    nc.sync.dma_start(
                        wrap_b[0:16, ph * bw:(ph + 1) * bw],
                        idx16[ph * 16:(ph + 1) * 16, :])
                # log2 replicate to all 8 groups of 16 partitions
                nc.sync.dma_start(wrap_b[16:32, :], wrap_b[0:16, :])
                nc.sync.dma_start(wrap_b[32:64, :], wrap_b[0:32, :])
                nc.sync.dma_start(wrap_b[64:128, :], wrap_b[0:64, :])

                # 16 ring-safe gathers of 1024 ids each
                zq_b = zqp.tile([128, BATCH_POS * D // 128], mybir.dt.float32,
                                tag="zq")
                NG = BATCH_POS // 1024                # 16
                for c in range(NG):
                    nc.gpsimd.dma_gather(
                        out_ap=zq_b[:, c * 512:(c + 1) * 512].rearrange(
                            "p (q d) -> p q d", d=D),
                        in_ap=cbf[:, :],
                        idxs_ap=wrap_b[:, c * 64:(c + 1) * 64],
                        num_idxs=1024,
                        num_idxs_reg=1024,
                        elem_size=D,
                    )
                nc.sync.dma_start(
                    zq_out[:, b * (BATCH_POS * D // 128):
                           (b + 1) * (BATCH_POS * D // 128)], zq_b[:, :])


def _get_nc():
    key = "default"
    if key not in _NC_CACHE:
        _NC_CACHE[key] = _build_bass()
    return _NC_CACHE[key]


def _prep_shared(codebook):
    cb64 = codebook.astype(np.float64)
    c_hi = codebook.astype(np.float16)                                   # [H,K,D]
    c_lo = (codebook - c_hi.astype(np.float32)).astype(np.float16)
    bias = -0.5 * (cb64 ** 2).sum(-1)                                    # [H,K] f64
    b_hi = bias.astype(np.float16)
    b_lo = (bias - b_hi.astype(np.float64)).astype(np.float16)

    rhs1 = np.empty((H, 66, K), np.float16)
    rhs2 = np.empty((H, 128, K), np.float16)
    for h in range(H):
        rhs1[h, :D] = c_hi[h].T
        rhs1[h, D] = b_hi[h]
        rhs1[h, D + 1] = b_lo[h]
        rhs2[h, :D] = c_lo[h].T
        rhs2[h, D:] = c_hi[h].T
    iota = np.broadcast_to(
        np.arange(K, dtype=np.float32)[None, :], (128, K)).copy()
    # per-tile-column head offsets (h*K), broadcast across partitions
    off = (np.arange(TILES) // TILES_PER_HEAD * K).astype(np.float32)
    goff = np.broadcast_to(off[None, :], (128, TILES)).copy()
    cbf = np.ascontiguousarray(codebook.reshape(H * K, D), dtype=np.float32)
    return rhs1, rhs2, iota, goff, cbf


def _prep_core(emb_shard):
    # emb_shard [NS, H, D] f32 -> za [128, H*NS] f16, zb [66, H*NS] f16
    a = np.ascontiguousarray(emb_shard.transpose(2, 1, 0))   # [D, H, NS]
    a2 = a.reshape(D, H * NS)
    z_hi = a2.astype(np.float16)
    z_lo = (a2 - z_hi.astype(np.float32)).astype(np.float16)
    za = np.concatenate([z_hi, z_lo], axis=0)                # [128, H*NS]
    zb = np.concatenate(
        [z_hi, np.ones((2, H * NS), np.float16)], axis=0)    # [66, H*NS]
    return za, zb


def _zq_perm():
    """Return (n_idx, h_idx) arrays decoding zq_out columns.

    zq_out region for batch b: [128, 128, 64] as (p', q, d) holding the
    codebook row of list position i = q*128 + p'; position i was written
    from idx16 slot (p = ph*16 + pl, t = colw % bw) with pl = i % 16,
    colw = i // 16, ph = colw // bw; that slot is row n = (t % 32)*128 + p
    of head h = b*HB + t // 32.
    """
    pp, qq = np.meshgrid(np.arange(128), np.arange(128), indexing="ij")
    i = qq * 128 + pp
    bw = HB * TILES_PER_HEAD
    pl = i % 16
    colw = i // 16
    ph = colw // bw
    t = colw % bw
    p = ph * 16 + pl
    n_local = (t % TILES_PER_HEAD) * 128 + p
    h_local = t // TILES_PER_HEAD
    return n_local, h_local


def kernel(embeds: np.ndarray, codebook: np.ndarray, _return_perf=False):
    embeds = np.asarray(embeds, dtype=np.float32)
    codebook = np.asarray(codebook, dtype=np.float32)

    rhs1, rhs2, iota, goff, cbf = _prep_shared(codebook)
    in_maps = []
    for i in range(NCORES):
        za, zb = _prep_core(embeds[i * NS:(i + 1) * NS])
        in_maps.append({"za": za, "zb": zb, "rhs1": rhs1, "rhs2": rhs2,
                        "iota": iota, "goff": goff, "cbf": cbf})

    nc = _get_nc()
    res = run_bass_kernel_spmd(nc, in_maps, core_ids=list(range(NCORES)))

    n_loc, h_loc = _zq_perm()
    quantized = np.empty((N, H, D), np.float32)
    idx = np.empty((N, H), np.int32)
    m_sum = 0.0
    for i in range(NCORES):
        r = res.results[i]
        for b in range(H // HB):
            zqb = r["zq_out"][:, b * 8192:(b + 1) * 8192].reshape(
                128, 128, D)                      # (p', q, d)
            quantized[i * NS + n_loc, b * HB + h_loc] = zqb
        ix = r["idx_out"].astype(np.int32).reshape(128, H, TILES_PER_HEAD)
        ix = ix - (np.arange(H) * K)[None, :, None]
        idx[i * NS:(i + 1) * NS] = ix.transpose(2, 0, 1).reshape(NS, H)
        m_sum += r["m_out"].astype(np.float64).sum()

    znorm_total = (embeds.astype(np.float64) ** 2).sum()
    mse = (znorm_total - 2.0 * m_sum) / (N * H * D)
    loss = np.float32((1.0 + BETA) * mse)

    if _return_perf:
        return (quantized, idx, loss), res
    return quantized, idx, loss
